# revision 1
# baseline (speedup 1.0000x reference)
"""LSG (local-sparse-global) block attention on 8 trn2 NeuronCores.

Shape/config facts hardcoded from the problem spec:
  n=2 h=12 t=4096 d=64, BLOCK=128, SPARSE_BLOCK=128, SF=4 -> ts=1024, g=64.
Per 128-token query block b the key set is:
  global (64, padded to 128) | sparse W1 [32b-160,32b-32) | sparse W2
  [32b+64,32b+192) | local [128(b-1), 128(b+2))            -> 704 real keys.

Sharding: n*h = 24 pairs, 3 per core (data parallel, no cross-core comm).

Device-side design (per pair, 32 blocks):
 - Host ships Q^T / K^T (d-major, 64 rows duplicated to 128 partitions so
   consecutive matmuls can run concurrently in the two PE row-groups, and
   DMA uses all SBUF ports).
 - Masking is folded into the value side: softmax(s+m) = exp(s)*exp(m)
   normalized, so the host scales each key's value row and the appended
   ones-column by exp(mask) (zero at structural pads). Zero-padded keys
   then produce exp(0)=1 junk probs that contribute exactly 0.
 - scoresT chunks (keys on partitions, queries free) via 6 K=64 matmuls
   per block, row-group alternating; one Exp per 2-block batch (bf16).
 - ctx matmuls: stationary = (values | exp(mask)) chunk (128, 65);
   moving = probsT chunk. One PSUM group accumulates ctx^T (64, q) and
   the softmax denominator (row 64).
 - ctx matmuls flip the operands (probsT chunk stationary via FWL,
   values moving) so ctx lands in natural (q, d) layout with per-partition
   denominators; normalize on DVE; contiguous full-partition stores.
"""

import numpy as np
import ml_dtypes
from contextlib import ExitStack

import concourse.bacc as bacc
import concourse.bass as bass
import concourse.tile as tile
from concourse import mybir
from concourse.bass_utils import run_bass_kernel_spmd
from concourse.tile import add_dep_helper

N, H, T, D = 2, 12, 4096, 64
TS, G = 1024, 64
NCORES = 8
PAIRS = (N * H) // NCORES  # 3
NBLK = T // 128            # 32
BF16 = ml_dtypes.bfloat16

LAST_RESULTS = None  # BassKernelResults of the most recent run (for test.py)


def build_program(pairs=PAIRS):
    dt = mybir.dt
    nc = bacc.Bacc("TRN2", target_bir_lowering=False, debug=False)

    qtb = nc.dram_tensor("qtb", [pairs, 128, T], dt.bfloat16, kind="ExternalInput").ap()
    ktb = nc.dram_tensor("ktb", [pairs, 128, T + 256], dt.bfloat16, kind="ExternalInput").ap()
    stb = nc.dram_tensor("stb", [pairs, 128, TS + 320], dt.bfloat16, kind="ExternalInput").ap()
    gtb = nc.dram_tensor("gtb", [pairs, 128, 128], dt.bfloat16, kind="ExternalInput").ap()
    vlb = nc.dram_tensor("vlb", [pairs, 128, 34, D + 1], dt.bfloat16, kind="ExternalInput").ap()
    svb = nc.dram_tensor("svb", [pairs, 128, 4, 10, D + 1], dt.bfloat16, kind="ExternalInput").ap()
    gvb = nc.dram_tensor("gvb", [pairs, 128, D + 1], dt.bfloat16, kind="ExternalInput").ap()
    oT = nc.dram_tensor("oT", [pairs, T, D], dt.float32, kind="ExternalOutput").ap()

    with tile.TileContext(nc) as tc, ExitStack() as ctx:
        big = ctx.enter_context(tc.tile_pool(name="big", bufs=2))
        probs = ctx.enter_context(tc.tile_pool(name="probs", bufs=3))
        outp = ctx.enter_context(tc.tile_pool(name="outp", bufs=3))
        ps_pool = ctx.enter_context(tc.tile_pool(name="psp", bufs=2, space="PSUM"))
        cx_pool = ctx.enter_context(tc.tile_pool(name="cxp", bufs=2, space="PSUM"))

        for p in range(pairs):
            # inputs alternate between the SP HWDGE queue and gpsimd's SWDGE
            eng_a = nc.sync
            eng_b = nc.sync
            qt = big.tile([128, T], dt.bfloat16, tag="qt")
            eng_a.dma_start(out=qt, in_=qtb[p])
            kt = big.tile([128, T + 256], dt.bfloat16, tag="kt")
            eng_b.dma_start(out=kt, in_=ktb[p])
            st = big.tile([128, TS + 320], dt.bfloat16, tag="st")
            eng_a.dma_start(out=st, in_=stb[p])
            gt = big.tile([128, 128], dt.bfloat16, tag="gt")
            eng_a.dma_start(out=gt, in_=gtb[p])
            vl = big.tile([128, 34, D + 1], dt.bfloat16, tag="vl")
            eng_b.dma_start(out=vl, in_=vlb[p])
            sv = big.tile([128, 4, 10, D + 1], dt.bfloat16, tag="sv")
            eng_a.dma_start(out=sv, in_=svb[p])
            gv = big.tile([128, D + 1], dt.bfloat16, tag="gv")
            eng_b.dma_start(out=gv, in_=gvb[p])

            for i in range(NBLK // 2):
                blocks = (2 * i, 2 * i + 1)
                # --- scoresT: 6 chunks x (128 keys, 128 queries) per block.
                # K=64; consecutive matmuls alternate PE row-groups (via the
                # operands' base partition) so they execute concurrently.
                # Segment-major order: (A-seg, B-seg) adjacent matmuls use
                # different PE row-groups AND land in different PSUM banks —
                # concurrent row-group matmuls into the same bank are fatal.
                ps = ps_pool.tile([128, 2, 768], dt.float32, tag="ps")
                stats = {}
                for bi, b in enumerate(blocks):
                    stat = [kt[:, (b + j) * 128:(b + j + 1) * 128] for j in range(3)]
                    stat.append(st[:, b * 32:b * 32 + 128])
                    stat.append(st[:, b * 32 + 224:b * 32 + 352])
                    stat.append(gt)
                    stats[bi] = stat
                prev = None
                for si in range(6):
                    for bi, b in enumerate(blocks):
                        rows = slice(bi * 64, bi * 64 + 64)
                        inst = nc.tensor.matmul(
                            ps[:, bi, si * 128:(si + 1) * 128],
                            stats[bi][si][rows, :],
                            qt[rows, b * 128:(b + 1) * 128],
                            start=True, stop=True)
                        if prev is not None:
                            add_dep_helper(inst.ins, prev.ins, sync=False)
                        prev = inst
                # --- probsT = exp(scoresT/8): one ACT instruction per batch
                pb = probs.tile([128, 2, 768], dt.bfloat16, tag="pb")
                nc.scalar.activation(pb, ps, mybir.ActivationFunctionType.Exp,
                                     scale=0.125)
                # --- ctx + denominator in natural (q, d) layout: stationary
                # is the probsT chunk (128x128 bf16, FWL), moving is the
                # (values | exp(mask)) chunk (128, 65). One PSUM group:
                # cols 0:64 = unnormalized ctx, col 64 = denominator.
                cx = cx_pool.tile([128, 2, D + 1], dt.float32, tag="cx")
                first_insts = {}
                last_inst = None
                for bi, b in enumerate(blocks):
                    ops = []
                    for j in range(3):
                        ops.append((pb[:, bi, j * 128:(j + 1) * 128], vl[:, b + j, :]))
                    ops.append((pb[:, bi, 384:512], sv[:, b % 4, b // 4, :]))
                    w2 = b + 7
                    ops.append((pb[:, bi, 512:640], sv[:, w2 % 4, w2 // 4, :]))
                    ops.append((pb[:, bi, 640:768], gv))
                    for oi, (lhsT, rhs) in enumerate(ops):
                        start = (bi == 0 and oi == 0)
                        stop = (bi == 1 and oi == len(ops) - 1)
                        inst = nc.tensor.matmul(cx[:, bi, :], lhsT, rhs,
                                                start=start, stop=stop)
                        if oi == 0:
                            first_insts[bi] = inst
                        last_inst_prev, last_inst = last_inst, inst
                # keep the single accumulation group well-ordered: the start
                # MM executes first, the stop MM last
                add_dep_helper(first_insts[1].ins, first_insts[0].ins, sync=False)
                add_dep_helper(last_inst.ins, last_inst_prev.ins, sync=False)
                # --- normalize on device and store contiguously
                rec = outp.tile([128, 2, 1], dt.float32, tag="rec")
                nc.vector.reciprocal(rec, cx[:, :, D:D + 1])
                ob = outp.tile([128, 2, D], dt.float32, tag="ob")
                for bi in range(2):
                    nc.vector.tensor_scalar_mul(ob[:, bi, :], cx[:, bi, 0:D],
                                                rec[:, bi, :])
                nc.sync.dma_start(
                    out=oT[p][i * 256:(i + 1) * 256, :].rearrange(
                        "(b q) d -> q b d", b=2),
                    in_=ob)

    nc.compile()
    return nc


def _prep_pair(q, k, v, am, sk, sv, sm, gk, gv, gm):
    """Build the device-layout arrays for one (n, h) pair. All inputs fp32
    numpy: q/k/v (T, D); am (T,); sk/sv (TS, D); sm (TS,); gk/gv (G, D);
    gm (G,). Returns dict of bf16 arrays."""
    def dup(x64):
        return np.concatenate([x64, x64], axis=0)

    def expm(mask_vals):
        # exp(mask): 1.0 for zero mask, 0.0 for -inf-ish masks
        with np.errstate(over="ignore", under="ignore"):
            return np.exp(np.minimum(mask_vals, 60.0)).astype(np.float32)

    qt = dup(q.T)

    kt = np.zeros((64, T + 256), np.float32)
    kt[:, 128:128 + T] = k.T
    kt = dup(kt)

    stm = np.zeros((64, TS + 320), np.float32)
    stm[:, 160:160 + TS] = sk.T
    stm = dup(stm)

    gt = np.zeros((64, 128), np.float32)
    gt[:, :G] = gk.T
    gt = dup(gt)

    # value side: row k scaled by exp(mask_k); appended col = exp(mask_k);
    # structural pads stay all-zero.
    em = expm(am)
    vpad = np.zeros((T + 256, D + 1), np.float32)
    vpad[128:128 + T, :D] = v * em[:, None]
    vpad[128:128 + T, D] = em
    vlb = vpad.reshape(34, 128, D + 1).transpose(1, 0, 2)

    esm = expm(sm)
    spad = np.zeros((TS + 320, D + 1), np.float32)
    spad[160:160 + TS, :D] = sv * esm[:, None]
    spad[160:160 + TS, D] = esm
    svb = np.zeros((128, 4, 10, D + 1), np.float32)
    for r in range(4):
        nj = 10 if r < 3 else 9
        for j in range(nj):
            svb[:, r, j] = spad[32 * r + 128 * j: 32 * r + 128 * j + 128]

    egm = expm(gm)
    gvb = np.zeros((128, D + 1), np.float32)
    gvb[:G, :D] = gv * egm[:, None]
    gvb[:G, D] = egm

    return dict(qtb=qt.astype(BF16), ktb=kt.astype(BF16), stb=stm.astype(BF16),
                gtb=gt.astype(BF16), vlb=vlb.astype(BF16), svb=svb.astype(BF16),
                gvb=gvb.astype(BF16))


def prep_inputs(inputs):
    """Full inputs -> list of per-core in_maps."""
    q = np.asarray(inputs["query_layer"], np.float32)
    k = np.asarray(inputs["key_layer"], np.float32)
    v = np.asarray(inputs["value_layer"], np.float32)
    am = np.asarray(inputs["attention_mask"], np.float32)[:, 0, 0, :]
    sk = np.asarray(inputs["sparse_key"], np.float32)
    sv = np.asarray(inputs["sparse_value"], np.float32)
    sm = np.asarray(inputs["sparse_mask"], np.float32)[:, 0, 0, :]
    gk = np.asarray(inputs["global_key"], np.float32)
    gv = np.asarray(inputs["global_value"], np.float32)
    gm = np.asarray(inputs["global_mask"], np.float32)[:, 0, 0, :]

    in_maps = []
    for c in range(NCORES):
        per_key = {}
        for pp in range(PAIRS):
            pair = c * PAIRS + pp
            n, h = divmod(pair, H)
            d = _prep_pair(q[n, h], k[n, h], v[n, h], am[n],
                           sk[n, h], sv[n, h], sm[n], gk[n, h], gv[n, h], gm[n])
            for name, arr in d.items():
                per_key.setdefault(name, []).append(arr)
        in_maps.append({name: np.stack(arrs) for name, arrs in per_key.items()})
    return in_maps


_prog_cache = {}


def _get_program():
    if "nc" not in _prog_cache:
        _prog_cache["nc"] = build_program()
    return _prog_cache["nc"]


def kernel(**inputs):
    global LAST_RESULTS
    nc = _get_program()
    in_maps = prep_inputs(inputs)
    res = run_bass_kernel_spmd(nc, in_maps, list(range(NCORES)))
    LAST_RESULTS = res
    out = np.empty((N, H, T, D), np.float32)
    for c in range(NCORES):
        oT = res.results[c]["oT"]  # (PAIRS, T, D)
        for pp in range(PAIRS):
            pair = c * PAIRS + pp
            n, h = divmod(pair, H)
            out[n, h] = oT[pp]
    return out

